# revision 24
# baseline (speedup 1.0000x reference)
"""Multi-head attention Trainium2 kernel (8 NeuronCores).

Problem: x[2,2048,1024] -> MHA(16 heads, d=64) -> out[2,2048,1024], fp32.

Sharding: 2-way data parallel on batch x 4-way tensor parallel on heads.
Core c handles batch c//4 and heads 4*(c%4)..4*(c%4)+3. Each core returns a
partial output [2048,1024]; the host sums the 4 TP partials per batch and
adds the separable bias terms (bo + bv@Wo; bk drops out of softmax).

On-core dataflow, organized so the Activation engine (which does all the
exp work, the hard floor at ~143us) never waits and every other engine
stays out of its way:
  - All matmuls run in bf16 (fp8 was tried for Q/K and measured 2.3e-2
    end-to-end error vs the 2e-2 gate; softmax needs bf16 inputs).
  - Attention loop per (pair, q-span of 512): S^T for 2 heads lands in one
    [128,1024] PSUM tile (one 64-row matmul per head, quadrant-packed);
    ONE exp covers both heads; PV is software-pipelined one kc behind exp.
  - All PSUM->SBUF copies run on DVE (none on the scalar engine).
  - Projection/V/Wo chains are interleaved as PE fillers inside the
    attention calls through a dedicated PSUM pool (banks: S=4, Oacc=2,
    fillers=2).
  - Softmax denominators come from a ones-row appended to V (row 65);
    normalization uses reciprocal_approx_fast + gpsimd partition broadcast
    (PE rank-1 ones-matmul broadcast for the final call, where PE is idle).
  - A short block of no-dependency warmup matmuls ramps the PE p-state to
    the full 2.4GHz clock while the input DMAs land.
"""

import numpy as np

B = 2
N = 2048
E = 1024
HEADS = 16
D = 64
P = 128
NCORES = 8
GROUPS = 4            # TP groups
DG = E // GROUPS      # 256 cols per core
ECH = E // P          # 8 contraction chunks
EPAIR = ECH // 2      # 4 DoubleRow pairs
NCH = N // P          # 16 key chunks
SP = 512              # q-span per attention call
NSPAN = N // SP       # 4 spans
WSCALE = 32.0         # fp8 weight prescale; folded into exp scale

_CACHE = {}


def _build():
    import sys
    if "/opt/trn_rl_repo" not in sys.path:
        sys.path.insert(0, "/opt/trn_rl_repo")
    import concourse.tile as tile
    from concourse import bacc, mybir
    from concourse.bass import ts

    F32 = mybir.dt.float32
    F8 = mybir.dt.float8e4
    BF16 = mybir.dt.bfloat16
    Exp = mybir.ActivationFunctionType.Exp
    DR = mybir.MatmulPerfMode.DoubleRow

    nc = bacc.Bacc("TRN2", target_bir_lowering=False, debug=False, num_devices=NCORES)

    # inputs are host-prearranged into SBUF layout for contiguous DMAs
    xt16c = [nc.dram_tensor(f"xt16c{c}", [P, ECH, SP], BF16, kind="ExternalInput").ap()
             for c in range(NSPAN)]
    wq16 = nc.dram_tensor("wq16", [P, ECH, DG], BF16, kind="ExternalInput").ap()
    wk16 = nc.dram_tensor("wk16", [P, ECH, DG], BF16, kind="ExternalInput").ap()
    wv16 = nc.dram_tensor("wv16", [P, ECH, DG], BF16, kind="ExternalInput").ap()
    wo16 = nc.dram_tensor("wo16", [P, 2, E], BF16, kind="ExternalInput").ap()
    bq2 = nc.dram_tensor("bq2", [P, 2], F32, kind="ExternalInput").ap()
    out = nc.dram_tensor("out", [N, E], F32, kind="ExternalOutput").ap()

    with tile.TileContext(nc) as tc:
        with tc.tile_pool(name="persist", bufs=1) as pers, \
             tc.tile_pool(name="pexp", bufs=4) as pexp_pool, \
             tc.tile_pool(name="small", bufs=2) as small, \
             tc.tile_pool(name="ostage", bufs=4) as ostage, \
             tc.tile_pool(name="psS", bufs=2, space="PSUM") as pS, \
             tc.tile_pool(name="psO", bufs=1, space="PSUM") as pO, \
             tc.tile_pool(name="psF", bufs=2, space="PSUM") as pF:
            wq16_sb = pers.tile([P, ECH, DG], BF16, tag="wq16")
            wk16_sb = pers.tile([P, ECH, DG], BF16, tag="wk16")
            wv16_sb = pers.tile([P, ECH, DG], BF16, tag="wv16")
            wo16_sb = pers.tile([P, 2, E], BF16, tag="wo16")
            bq_sb = pers.tile([P, 2], F32, tag="bq")
            xt16_sb = pers.tile([P, ECH, N], BF16, tag="xt16")
            kT_p = [pers.tile([P, N], BF16, tag=f"kT{i}", name=f"kT{i}") for i in range(2)]
            qT_p = [pers.tile([P, N], BF16, tag=f"qT{i}", name=f"qT{i}") for i in range(2)]
            v_sb = pers.tile([P, NCH, GROUPS, 66], BF16, tag="v")
            oT_sb = pers.tile([P, 2, N], BF16, tag="oT")

            def kq_chain(pair, w_sb, dst, bias, s):
                a, b = kq_chain2(pair, w_sb, dst, bias, s)
                def emit():
                    a()
                    b()
                return emit

            def kq_chain2(pair, w_sb, dst, bias, s):
                """The projection chain split into two fillers so a single
                pop never stalls the exp pipeline by more than ~0.9us."""
                cell = {}
                def half(lo, hi):
                    def emit():
                        if lo == 0:
                            cell["ps"] = pF.tile([P, SP], F32, tag="F",
                                                 name=f"kq{pair}{s}")
                        ps = cell["ps"]
                        for ec in range(lo, hi):
                            nc.tensor.matmul(
                                ps,
                                w_sb[:, ec, ts(pair, P)],
                                xt16_sb[:, ec, ts(s, SP)],
                                start=(ec == 0), stop=(ec == ECH - 1),
                            )
                        if hi == ECH:
                            if bias:
                                nc.vector.tensor_add(
                                    dst[:, ts(s, SP)], ps,
                                    bq_sb[:, pair, None].to_broadcast((P, SP)),
                                )
                            else:
                                nc.vector.tensor_copy(dst[:, ts(s, SP)], ps)
                    return emit
                return half(0, ECH // 2), half(ECH // 2, ECH)

            def v_chain(ncx, pair):
                def emit():
                    ps = pF.tile([P, SP], F32, tag="F", name=f"v{ncx}{pair}")
                    psl = ps[:, :P]
                    for ec in range(ECH):
                        nc.tensor.matmul(
                            psl,
                            xt16_sb[:, ec, ts(ncx, P)],
                            wv16_sb[:, ec, ts(pair, P)],
                            start=(ec == 0), stop=(ec == ECH - 1),
                        )
                    nc.vector.tensor_copy(
                        v_sb[:, ncx, 2 * pair:2 * pair + 2, 0:64],
                        psl.rearrange("p (h d) -> p h d", d=D),
                    )
                return emit

            def wo_chain2(ncx, on_scalar=False):
                cell = {}
                def unit(fb):
                    def emit():
                        if fb == 0:
                            cell["ot"] = ostage.tile([P, 2 * SP], F32, tag="ot",
                                                     name=f"ot{ncx}")
                        ot = cell["ot"]
                        ps = pF.tile([P, SP], F32, tag="F", name=f"wo{ncx}{fb}")
                        for dc in range(2):
                            nc.tensor.matmul(
                                ps,
                                oT_sb[:, dc, ts(ncx, P)],
                                wo16_sb[:, dc, ts(fb, SP)],
                                start=(dc == 0), stop=(dc == 1),
                            )
                        if on_scalar:
                            nc.scalar.copy(ot[:, ts(fb, SP)], ps)
                        else:
                            nc.vector.tensor_copy(ot[:, ts(fb, SP)], ps)
                        if fb == 1:
                            (nc.scalar if on_scalar else nc.sync).dma_start(
                                out[ts(ncx, P), :], ot)
                    return emit
                return unit(0), unit(1)

            def wo_chain(ncx, on_scalar=False):
                a, b = wo_chain2(ncx, on_scalar)
                def emit():
                    a()
                    b()
                return emit

            def pv_step(st, kcp):
                pair, qlen = st["pair"], st["qlen"]
                pe = st["pes"][kcp]
                for h in range(2):
                    nc.tensor.matmul(
                        st["oaccs"][h][:, :qlen],
                        v_sb[:, kcp, 2 * pair + h, 0:65],
                        pe[:, h * qlen:(h + 1) * qlen],
                        start=(kcp == 0), stop=(kcp == NCH - 1),
                    )

            def norm(st, pe_bcast=False):
                # normalize: O = O' * (1/denom), denom in oacc row 64.
                # Denominator rows are first copied to partition-0-based
                # tiles: reciprocal/broadcast inputs must sit at base 0.
                pair, q0, qlen = st["pair"], st["q0"], st["qlen"]
                osps = []
                for h in range(2):
                    osp = small.tile([65, SP], F32, tag=f"osp{h}", name="osp")
                    nc.vector.tensor_copy(osp[:, :qlen], st["oaccs"][h][:, :qlen])
                    osps.append(osp)
                dn = small.tile([33, SP], F32, tag="dn", name="dn")
                for h in range(2):
                    nc.vector.tensor_copy(dn[32 * h:32 * h + 1, :qlen], osps[h][64:65, :qlen])
                rec = small.tile([33, SP], F32, tag="rec", name="rec")
                nc.vector.reciprocal_approx_fast(rec[:, :qlen], dn[:, :qlen])
                rv1 = small.tile([1, SP], F32, tag="rv1", name="rv1")
                nc.vector.tensor_copy(rv1[:, :qlen], rec[32:33, :qlen])
                for h in range(2):
                    if pe_bcast:
                        # final call: PE is idle, broadcast via rank-1 matmul
                        rb = small.tile([1, SP], BF16, tag=f"rb{h}", name="rb")
                        nc.vector.tensor_copy(
                            rb[:, :qlen], rec[32 * h:32 * h + 1, :qlen])
                        rbc = pF.tile([P, SP], F32, tag="F", name="rbc")
                        nc.tensor.matmul(
                            rbc[0:64, :qlen], ones_bf[0:1, :], rb[:, :qlen],
                            start=True, stop=True,
                        )
                    else:
                        rbc = small.tile([P, SP], F32, tag="rbc", name="rbc")
                        nc.gpsimd.partition_broadcast(
                            rbc[:, :qlen], rec[0:1, :qlen] if h == 0 else rv1[:, :qlen])
                    nc.vector.tensor_mul(
                        oT_sb[64 * h:64 * h + 64, pair, q0:q0 + qlen],
                        osps[h][0:64, :qlen],
                        rbc[0:64, :qlen],
                    )

            def emit_attn_seq(calls):
                """Flat software pipeline across all attention calls: PV lags
                exp by one iteration even across call boundaries, so the PE
                never waits at a boundary; norm of call c is emitted inside
                call c+1's first iteration."""
                prev = None  # (state, kcp) pending PV
                states = []
                for pair, q0, qlen, fillers in calls:
                    st = {
                        "pair": pair, "q0": q0, "qlen": qlen,
                        "oaccs": [pO.tile([65, SP], F32, tag=f"O{h}", name=f"oacc{h}")
                                  for h in range(2)],
                        "pes": {},
                    }
                    states.append(st)
                    fillers = list(fillers)
                    for kc in range(NCH):
                        if kc >= 1 and fillers:
                            u = fillers.pop(0)
                            for f in (u if isinstance(u, tuple) else (u,)):
                                f()
                        # h-halves at bank-aligned offsets h*SP (separate
                        # accumulation banks even for qlen<SP)
                        pss = pS.tile([P, 2 * SP], F32, tag="S", name="spsum")
                        for h in range(2):
                            nc.tensor.matmul(
                                pss[:, h * SP:h * SP + qlen],
                                kT_p[pair][64 * h:64 * h + 64, ts(kc, P)],
                                qT_p[pair][64 * h:64 * h + 64, q0:q0 + qlen],
                                start=True, stop=True,
                            )
                        pe = pexp_pool.tile([P, 2 * SP], BF16, tag="pexp", name="pexp")
                        nc.scalar.activation(
                            pe[:, :2 * qlen].rearrange("p (h q) -> p h q", h=2),
                            pss.rearrange("p (h q) -> p h q", h=2)[:, :, 0:qlen],
                            Exp, scale=1.0 / 32.0,
                        )
                        st["pes"][kc] = pe
                        if prev is not None:
                            pst, pkc = prev
                            pv_step(pst, pkc)
                            if pkc == NCH - 1:
                                norm(pst)
                                pst["pes"].clear()
                        prev = (st, kc)
                    for f in fillers:
                        f()
                # drain the last pending PV + norm (PE broadcast: PE idle here)
                pst, pkc = prev
                pv_step(pst, pkc)
                norm(pst, pe_bcast=True)

            # ---- input DMAs ----
            nc.scalar.dma_start(wk16_sb, wk16)
            nc.scalar.dma_start(wq16_sb, wq16)
            nc.scalar.dma_start(bq_sb, bq2)
            nc.sync.dma_start(xt16_sb[:, :, ts(0, SP)], xt16c[0])
            nc.scalar.dma_start(wv16_sb, wv16)
            nc.sync.dma_start(xt16_sb[:, :, ts(1, SP)], xt16c[1])
            nc.scalar.dma_start(xt16_sb[:, :, ts(2, SP)], xt16c[2])
            nc.sync.dma_start(xt16_sb[:, :, ts(3, SP)], xt16c[3])
            nc.scalar.dma_start(wo16_sb, wo16)

            ones_f32 = pers.tile([P, 1], F32, tag="ones")
            nc.vector.memset(ones_f32, 1.0)
            ones_bf = pers.tile([1, 64], BF16, tag="onesb")
            nc.vector.memset(ones_bf, 1.0)
            nc.vector.tensor_copy(
                v_sb[:, :, :, 64:65],
                ones_f32[:, 0, None, None, None].to_broadcast((P, NCH, GROUPS, 1)),
            )

            # ---- PE warmup: no-dependency matmuls run while the input DMAs
            # land, ramping the PE p-state to full clock (idle PE drops to
            # 1.2GHz; full 2.4GHz needs ~3us of continuous execution).
            wu = pers.tile([P, SP], BF16, tag="warm")
            nc.vector.memset(wu, 0.0)
            for w in range(26):
                wps = pS.tile([P, 2 * SP], F32, tag="S", name="warm")
                nc.tensor.matmul(wps[:, :SP], wu[:, :P], wu, start=True, stop=True)

            # ---- minimal upfront: K0/Q0 span 0, V pair-0 ncx 0-3 ----
            kq_chain(0, wk16_sb, kT_p[0], False, 0)()
            kq_chain(0, wq16_sb, qT_p[0], True, 0)()

            # ---- attention: one flat pipelined sequence (pair 0 first) ----
            emit_attn_seq([
                # A=(0,0): finish K0 + V pair-0 just ahead of their consumers
                (0, 0, SP, (lambda k1, k2, k3, q1: [
                    (v_chain(0, 0), v_chain(1, 0)),
                    (v_chain(2, 0), k1[0]),
                    (v_chain(3, 0), k1[1]),
                    v_chain(4, 0), v_chain(5, 0),
                    (v_chain(6, 0), k2[0]),
                    (v_chain(7, 0), k2[1]),
                    v_chain(8, 0), v_chain(9, 0),
                    (v_chain(10, 0), k3[0]),
                    (v_chain(11, 0), k3[1]),
                    (v_chain(12, 0), q1[0]),
                    (v_chain(13, 0), q1[1]),
                    v_chain(14, 0), v_chain(15, 0),
                ])(kq_chain2(0, wk16_sb, kT_p[0], False, 1), kq_chain2(0, wk16_sb, kT_p[0], False, 2), kq_chain2(0, wk16_sb, kT_p[0], False, 3), kq_chain2(0, wq16_sb, qT_p[0], True, 1))),
                # B=(0,1): start pair-1 K, first pair-1 V chains
                (0, 512, SP, [
                    *kq_chain2(0, wq16_sb, qT_p[0], True, 2),
                    *kq_chain2(1, wk16_sb, kT_p[1], False, 0),
                    *kq_chain2(1, wk16_sb, kT_p[1], False, 1),
                    v_chain(0, 1), v_chain(1, 1), v_chain(2, 1), v_chain(3, 1),
                ]),
                # C=(0,2)
                (0, 1024, SP, [
                    *kq_chain2(0, wq16_sb, qT_p[0], True, 3),
                    *kq_chain2(1, wk16_sb, kT_p[1], False, 2),
                    *kq_chain2(1, wk16_sb, kT_p[1], False, 3),
                    v_chain(4, 1), v_chain(5, 1), v_chain(6, 1), v_chain(7, 1),
                    v_chain(8, 1),
                ]),
                # D=(0,3)
                (0, 1536, SP, [
                    *kq_chain2(1, wq16_sb, qT_p[1], True, 0),
                    v_chain(9, 1), v_chain(10, 1), v_chain(11, 1), v_chain(12, 1),
                    v_chain(13, 1), v_chain(14, 1), v_chain(15, 1),
                ]),
                # E=(1,0)
                (1, 0, SP, [
                    *kq_chain2(1, wq16_sb, qT_p[1], True, 1),
                ]),
                # F=(1,1): wo for q-span 0 is ready (both pairs done)
                (1, 512, SP, [
                    *kq_chain2(1, wq16_sb, qT_p[1], True, 2),
                    *wo_chain2(0), *wo_chain2(1), *wo_chain2(2), *wo_chain2(3),
                ]),
                # G=(1,2)
                (1, 1024, SP, [
                    *kq_chain2(1, wq16_sb, qT_p[1], True, 3),
                    *wo_chain2(4), *wo_chain2(5), *wo_chain2(6), *wo_chain2(7),
                ]),
                # H=(1,3)
                (1, 1536, SP, [
                    *wo_chain2(8), *wo_chain2(9), *wo_chain2(10), *wo_chain2(11),
                ]),
            ])
            for ncx in range(12, 16):
                a, b = wo_chain2(ncx, on_scalar=True)
                a()
                b()

    nc.compile()
    return nc


def _get_nc():
    if "nc" not in _CACHE:
        _CACHE["nc"] = _build()
    return _CACHE["nc"]


def kernel(x, Wq, bq, Wk, bk, Wv, bv, Wo, bo, **run_kwargs):
    import sys
    if "/opt/trn_rl_repo" not in sys.path:
        sys.path.insert(0, "/opt/trn_rl_repo")
    import ml_dtypes
    from concourse.bass_utils import run_bass_kernel_spmd

    f8 = ml_dtypes.float8_e4m3
    bf16 = ml_dtypes.bfloat16

    x = np.asarray(x, dtype=np.float32)
    Wq = np.asarray(Wq, dtype=np.float32)
    Wk = np.asarray(Wk, dtype=np.float32)
    Wv = np.asarray(Wv, dtype=np.float32)
    Wo = np.asarray(Wo, dtype=np.float32)
    bq = np.asarray(bq, dtype=np.float32)
    bv = np.asarray(bv, dtype=np.float32)
    bo = np.asarray(bo, dtype=np.float32)

    nc = _get_nc()

    def _arr(t, inner):
        # [E, inner] -> [P, ECH, inner] in (c p) order
        return np.ascontiguousarray(
            t.reshape(ECH, P, t.shape[1]).transpose(1, 0, 2))

    xt16s = [_arr(np.ascontiguousarray(x[b].T).astype(bf16), N) for b in range(B)]
    in_maps = []
    for c in range(NCORES):
        b, g = divmod(c, GROUPS)
        cols = slice(g * DG, (g + 1) * DG)
        im = {
            f"xt16c{c}": np.ascontiguousarray(xt16s[b][:, :, c * SP:(c + 1) * SP])
            for c in range(NSPAN)
        }
        im.update({
            "wq16": _arr(Wq[:, cols].astype(bf16), DG),
            "wk16": _arr(Wk[:, cols].astype(bf16), DG),
            "wv16": _arr(Wv[:, cols].astype(bf16), DG),
            "wo16": np.ascontiguousarray(
                Wo[cols, :].astype(bf16).reshape(2, P, E).transpose(1, 0, 2)),
            "bq2": np.ascontiguousarray(bq[cols].reshape(2, P).T),
        })
        in_maps.append(im)

    try:
        res = run_bass_kernel_spmd(nc, in_maps, core_ids=list(range(NCORES)), **run_kwargs)
    except Exception:
        # device may be wedged from a prior run; reset the accelerator once
        try:
            import ctypes
            lib = ctypes.CDLL("/opt/axon/libaxon_pjrt.so")
            lib.axon_reset.restype = ctypes.c_int
            lib.axon_reset()
        except Exception:
            pass
        res = run_bass_kernel_spmd(nc, in_maps, core_ids=list(range(NCORES)), **run_kwargs)
    if run_kwargs:
        _CACHE["last_results"] = res

    # gather: sum TP partials per batch, add separable bias terms
    bias_vec = bv @ Wo + bo  # softmax rows sum to 1 => bv contributes bv@Wo
    full = np.empty((B, N, E), dtype=np.float32)
    for b in range(B):
        acc = res.results[b * GROUPS]["out"].astype(np.float32).copy()
        for g in range(1, GROUPS):
            acc += res.results[b * GROUPS + g]["out"]
        full[b] = acc + bias_vec[None, :]
    return full
